# revision 15
# baseline (speedup 1.0000x reference)
"""DeBERTa DisentangledSelfAttention forward on 8 Trainium2 NeuronCores.

Full inputs in, full outputs out.  Sharding: core c handles batch b = c // 4
and heads [4*(c%4), 4*(c%4)+4) -- 4 (b, h) pairs per core, SPMD (identical
program, per-core input data).

Math (per b,h), verified vs reference in numpy:
  scores[i,j] = scale * q_i . k_j
  c2p[i,j]    = scale * q_i . pos_k[Fc(i-j)]     (Fc = clip(bucket+K))
  p2c[i,j]    = scale * k_j . pos_q[Fp(j-i)]     (Fp = clip(-bucket+K))
Both relative terms are Toeplitz gathers.  With E_c[t'] = pos_k[Fc(1023-t')]
(t' in [0,2047)) and C_r[i,t'] = scale * q_i . E_c[t']:
  c2p[i,j] = C_r[i, 1023 - i + j]
i.e. row i of c2p is a contiguous window of row i of C_r starting at 1023-i.
That window read is a perfectly strided (overlapping) access over a flat DRAM
copy of C_r: flat[i*1151 + 127 + j] for band-limited storage (see below).
Same for p2c with (q,Ec) -> (k,Ep), producing p2c^T, which is folded back into
the logits PSUM accumulation with TensorE transpose matmuls.
"""

import math
import os
import sys

import numpy as np
import ml_dtypes

sys.path.insert(0, "/opt/trn_rl_repo")

import concourse.bass as bass  # noqa: E402
import concourse.bacc as bacc  # noqa: E402
import concourse.mybir as mybir  # noqa: E402
import concourse.tile as tile  # noqa: E402
from concourse.bass_utils import run_bass_kernel_spmd  # noqa: E402
from contextlib import ExitStack  # noqa: E402

F32 = mybir.dt.float32
F32R = mybir.dt.float32r
BF16 = mybir.dt.bfloat16
U16 = mybir.dt.uint16
AF = mybir.ActivationFunctionType
AX = mybir.AxisListType

B, S, H, D = 2, 1024, 16, 64
HID = H * D
K = 256
HPC = 4            # heads per core
NCORES = 8
NB = S // 128      # 8 s-blocks
BAND = 1152        # band columns stored per i-block (1151 needed + 1 pad)
EL = 2048          # expanded bucket length (2047 + 1 pad)
SCALE = 1.0 / math.sqrt(D * 3)

_COMPILED = None   # compiled Bacc cache
LAST_RESULT = None  # BassKernelResults of the most recent run (for profiling)


# --------------------------------------------------------------------------
# host-side index preparation
# --------------------------------------------------------------------------

def _delta_table(rp: np.ndarray) -> np.ndarray:
    """F_delta[d + 1023] = bucket(d) for d in [-1023, 1023], from rel pos."""
    fd = np.zeros(2 * S - 1, dtype=np.int64)
    d = np.arange(S)
    fd[(S - 1) + d] = rp[d, 0]
    fd[(S - 1) - d] = rp[0, d]
    return fd


def _wrap_idx(m: np.ndarray) -> np.ndarray:
    """[EL] index map -> [128, EL//16] uint16 in IndirectCopy wrapped layout."""
    ncols = EL // 16
    out = np.zeros((128, ncols), dtype=np.uint16)
    p = np.arange(128)
    s = np.arange(ncols)
    out[:, :] = m[(s[None, :] * 16 + (p[:, None] % 16))]
    return out


# --------------------------------------------------------------------------
# device kernel
# --------------------------------------------------------------------------

def _build_kernel():
    nc = bacc.Bacc("TRN2", target_bir_lowering=False, debug=False,
                   num_devices=NCORES)

    din = {}
    for name, shape, dt in [
        ("hsT", [HID, S], BF16),
        ("wqT", [HID, 256], BF16),
        ("wkT", [HID, 256], BF16),
        ("wvT", [HID, 256], BF16),
        ("bqs", [128, 2], F32),
        ("bk2", [128, 2], F32),
        ("bv1", [128, 256], F32),
        ("rembT", [HID, 2 * K], BF16),
        ("idxc", [128, EL // 16], U16),
        ("idxp", [128, EL // 16], U16),
        ("ident", [128, 128], BF16),
    ]:
        din[name] = nc.dram_tensor(name, shape, dt, kind="ExternalInput").ap()

    logits_o = nc.dram_tensor("logits_o", [HPC, S, S], F32,
                              kind="ExternalOutput").ap()
    probs_o = nc.dram_tensor("probs_o", [HPC, S, S], F32,
                             kind="ExternalOutput").ap()
    ctx_o = nc.dram_tensor("ctx_o", [HPC, S, D], F32,
                           kind="ExternalOutput").ap()

    # internal DRAM band scratch, one pair (C and P sides) per head
    bandC = [nc.dram_tensor(f"bandC{h}", [S * BAND], BF16) for h in range(HPC)]
    bandP = [nc.dram_tensor(f"bandP{h}", [S * BAND], BF16) for h in range(HPC)]

    with tile.TileContext(nc) as tc, ExitStack() as ctx:
        cst = ctx.enter_context(tc.tile_pool(name="cst", bufs=1))
        p2cp = ctx.enter_context(tc.tile_pool(name="p2cp", bufs=2))
        sb_band = ctx.enter_context(tc.tile_pool(name="sb_band", bufs=3))
        sb_skew = ctx.enter_context(tc.tile_pool(name="sb_skew", bufs=3))
        sb_lo = ctx.enter_context(tc.tile_pool(name="sb_lo", bufs=3))
        sb_exp = ctx.enter_context(tc.tile_pool(name="sb_exp", bufs=3))
        sb_small = ctx.enter_context(tc.tile_pool(name="sb_small", bufs=4))
        psL = ctx.enter_context(tc.tile_pool(name="psL", bufs=3, space="PSUM"))
        psS = ctx.enter_context(tc.tile_pool(name="psS", bufs=2, space="PSUM"))

        # ---------------- persistent SBUF tensors ----------------
        ident = cst.tile([128, 128], BF16)
        nc.sync.dma_start(ident[:], din["ident"])
        bqs = cst.tile([128, 2], F32)
        nc.sync.dma_start(bqs[:], din["bqs"])
        bk2 = cst.tile([128, 2], F32)
        nc.sync.dma_start(bk2[:], din["bk2"])
        bv1 = cst.tile([128, 256], F32)
        nc.sync.dma_start(bv1[:], din["bv1"])
        idxc = cst.tile([128, EL // 16], U16)
        nc.sync.dma_start(idxc[:], din["idxc"])
        idxp = cst.tile([128, EL // 16], U16)
        nc.sync.dma_start(idxp[:], din["idxp"])

        qTs = cst.tile([128, 2 * S], BF16)      # [d%128, pb*1024 + s], scaled
        kT = cst.tile([128, 2 * S], BF16)
        vbf = cst.tile([128, 2048], BF16)      # [s%128, sb*256 + dd]
        poskT = cst.tile([128, 1024], F32)     # [d%128, pb*512 + bucket]
        posqTs = cst.tile([128, 1024], F32)    # scaled
        EcT = cst.tile([128, 2 * EL], BF16)     # [d%128, pb*2048 + t']
        EpTs = cst.tile([128, 2 * EL], BF16)

        # ---------------- phase 0: projections ----------------
        with ExitStack() as c0:
            ph0 = c0.enter_context(tc.tile_pool(name="ph0", bufs=1))
            hsT = ph0.tile([128, 8 * S], BF16)
            nc.sync.dma_start(
                hsT[:].rearrange("p (a s) -> p a s", a=8),
                din["hsT"].rearrange("(a p) s -> p a s", p=128))
            wT = {}
            for w in ("wqT", "wkT", "wvT"):
                t = ph0.tile([128, 8 * 256], BF16, tag=w)
                nc.sync.dma_start(
                    t[:].rearrange("p (a d) -> p a d", a=8),
                    din[w].rearrange("(a p) d -> p a d", p=128))
                wT[w] = t
            rembT = ph0.tile([128, 8 * 512], BF16)
            nc.sync.dma_start(
                rembT[:].rearrange("p (a d) -> p a d", a=8),
                din["rembT"].rearrange("(a p) d -> p a d", p=128))

            # qT / kT: [d, s] layouts
            for dst, w, bias, scl in ((qTs, "wqT", bqs, SCALE),
                                      (kT, "wkT", bk2, 1.0)):
                for pb in range(2):
                    ps = psL.tile([128, 1024], F32, tag="L")
                    for kh in range(8):
                        for nh in range(2):
                            nc.tensor.matmul(
                                ps[:, nh * 512:(nh + 1) * 512],
                                lhsT=wT[w][:, kh * 256 + pb * 128:
                                           kh * 256 + pb * 128 + 128],
                                rhs=hsT[:, kh * 1024 + nh * 512:
                                        kh * 1024 + nh * 512 + 512
                                        ],
                                start=(kh == 0), stop=(kh == 7))
                    nc.scalar.activation(dst[:, pb * 1024:(pb + 1) * 1024],
                                         ps[:], AF.Identity,
                                         bias=bias[:, pb:pb + 1], scale=scl)

            # v: [s, d] layout, cast to bf16 with bias add
            for sb in range(8):
                ps = psS.tile([128, 256], F32, tag="S")
                for kh in range(8):
                    nc.tensor.matmul(
                        ps[:],
                        lhsT=hsT[:, kh * 1024 + sb * 128:
                                 kh * 1024 + sb * 128 + 128],
                        rhs=wT["wvT"][:, kh * 256:(kh + 1) * 256
                                      ],
                        start=(kh == 0), stop=(kh == 7))
                nc.vector.tensor_tensor(
                    out=vbf[:, sb * 256:(sb + 1) * 256],
                    in0=ps[:], in1=bv1[:], op=mybir.AluOpType.add)

            # positional projections: [d, bucket] layouts
            for dst, w, bias, scl in ((poskT, "wkT", bk2, 1.0),
                                      (posqTs, "wqT", bqs, SCALE)):
                for pb in range(2):
                    ps = psS.tile([128, 512], F32, tag="S")
                    for kh in range(8):
                        nc.tensor.matmul(
                            ps[:],
                            lhsT=wT[w][:, kh * 256 + pb * 128:
                                       kh * 256 + pb * 128 + 128
                                       ],
                            rhs=rembT[:, kh * 512:(kh + 1) * 512
                                      ],
                            start=(kh == 0), stop=(kh == 7))
                    nc.scalar.activation(dst[:, pb * 512:(pb + 1) * 512],
                                         ps[:], AF.Identity,
                                         bias=bias[:, pb:pb + 1], scale=scl)

        # bucket expansion along free axis (same map for every partition).
        # IndirectCopy is 4-byte only: gather in f32, then cast to bf16.
        with ExitStack() as ce:
            est = ce.enter_context(tc.tile_pool(name="est", bufs=2))
            for pb in range(2):
                for src_, idx_, dst_, eng in (
                        (poskT, idxc, EcT, nc.vector),
                        (posqTs, idxp, EpTs, nc.scalar)):
                    stg = est.tile([128, EL], F32)
                    for o in range(0, EL, 512):
                        nc.gpsimd.indirect_copy(
                            stg[:, o:o + 512],
                            src_[:, pb * 512:(pb + 1) * 512],
                            idx_[:, o // 16:(o + 512) // 16], True)
                    if eng is nc.vector:
                        nc.vector.tensor_copy(
                            dst_[:, pb * EL:(pb + 1) * EL], stg[:])
                    else:
                        nc.scalar.copy(dst_[:, pb * EL:(pb + 1) * EL], stg[:])

        # ---------------- per-pair phases ----------------
        for h in range(HPC):
            pb, pr = h // 2, 64 * (h % 2)
            qh = qTs[pr:pr + 64, pb * 1024:(pb + 1) * 1024]
            kh_ = kT[pr:pr + 64, pb * 1024:(pb + 1) * 1024]
            EcTh = EcT[pr:pr + 64, pb * EL:(pb + 1) * EL]
            EpTh = EpTs[pr:pr + 64, pb * EL:(pb + 1) * EL]

            # (a) band matmuls -> DRAM
            for src, Eh, dst in ((qh, EcTh, bandC[h]), (kh_, EpTh, bandP[h])):
                for ib in range(NB):
                    c0_ = 896 - 128 * ib
                    pA = psL.tile([128, 1024], F32, tag="L")
                    for nh in range(2):
                        nc.tensor.matmul(
                            pA[:, nh * 512:(nh + 1) * 512],
                            lhsT=src[:, ib * 128:ib * 128 + 128
                                     ],
                            rhs=Eh[:, c0_ + nh * 512:c0_ + nh * 512 + 512
                                   ],
                            start=True, stop=True)
                    pB = psS.tile([128, 128], F32, tag="S")
                    nc.tensor.matmul(
                        pB[:],
                        lhsT=src[:, ib * 128:ib * 128 + 128],
                        rhs=Eh[:, c0_ + 1024:c0_ + 1152],
                        start=True, stop=True)
                    sbB = sb_band.tile([128, BAND], BF16)
                    if ib % 2 == 0:
                        nc.scalar.copy(sbB[:, 0:1024], pA[:])
                        nc.vector.tensor_copy(sbB[:, 1024:1152], pB[:])
                    else:
                        nc.vector.tensor_copy(sbB[:, 0:1024], pA[:])
                        nc.scalar.copy(sbB[:, 1024:1152], pB[:])
                    nc.scalar.dma_start(
                        bass.AP(dst, ib * 128 * BAND,
                                [[BAND, 128], [1, BAND]]),
                        sbB[:])

            # (c) skewed window reads of P band -> p2c^T in SBUF
            p2cT = p2cp.tile([128, 8 * S], BF16)
            for jb in range(NB):
                nc.sync.dma_start(
                    p2cT[:, jb * S:(jb + 1) * S],
                    bass.AP(bandP[h], jb * 128 * BAND + 127,
                            [[BAND - 1, 128], [1, S]]))

            # (d) logits / softmax / PV per i-block
            ctxps = psS.tile([128, 512], F32, tag="S")
            for ib in range(NB):
                pL = psL.tile([128, 1024], F32, tag="L")
                for nh in range(2):
                    nc.tensor.matmul(
                        pL[:, nh * 512:(nh + 1) * 512],
                        lhsT=qh[:, ib * 128:ib * 128 + 128],
                        rhs=kh_[:, nh * 512:(nh + 1) * 512],
                        start=True, stop=False)
                c2ps = sb_skew.tile([128, S], BF16)
                nc.sync.dma_start(
                    c2ps[:],
                    bass.AP(bandC[h], ib * 128 * BAND + 127,
                            [[BAND - 1, 128], [1, S]]))
                for nh in range(2):
                    nc.tensor.matmul(pL[:, nh * 512:(nh + 1) * 512],
                                     lhsT=ident[:],
                                     rhs=c2ps[:, nh * 512:(nh + 1) * 512],
                                     start=False, stop=False)
                for jc in range(NB):
                    # regular matmul with lhsT=data, rhs=I == transpose of the
                    # bf16 block, accumulated into the f32 logits PSUM
                    nc.tensor.matmul(
                        pL[:, jc * 128:(jc + 1) * 128],
                        lhsT=p2cT[:, jc * S + ib * 128:jc * S + ib * 128 + 128],
                        rhs=ident[:],
                        start=False, stop=(jc == 3 or jc == 7))

                negmax = sb_small.tile([128, 1], F32)
                nc.vector.reduce_max(negmax[:], pL[:], axis=AX.X, negate=True)
                lo = sb_lo.tile([128, S], F32)
                nc.scalar.activation(lo[:], pL[:], AF.Identity,
                                     bias=negmax[:], scale=1.0)
                nc.scalar.dma_start(
                    logits_o[h, ib * 128:(ib + 1) * 128, :], lo[:])

                ex = sb_exp.tile([128, S], BF16, tag="exp")
                rsum = sb_small.tile([128, 1], F32)
                nc.scalar.activation(ex[:], lo[:], AF.Exp, accum_out=rsum[:])
                rs = sb_small.tile([128, 1], F32)
                nc.vector.reciprocal(rs[:], rsum[:])
                pr_sb = sb_lo.tile([128, S], F32, tag="probs")
                nc.scalar.activation(pr_sb[:], ex[:], AF.Copy, scale=rs[:])
                nc.scalar.dma_start(
                    probs_o[h, ib * 128:(ib + 1) * 128, :], pr_sb[:])

                pT = psL.tile([128, 1024], F32, tag="L")
                for jc in range(NB):
                    nc.tensor.matmul(
                        pT[:, jc * 128:(jc + 1) * 128],
                        lhsT=ex[:, jc * 128:(jc + 1) * 128],
                        rhs=ident[:],
                        start=True, stop=True)
                exT = sb_exp.tile([128, S], BF16, tag="expT")
                nc.vector.tensor_copy(exT[:], pT[:])
                for jc in range(NB):
                    nc.tensor.matmul(
                        ctxps[:, ib * 64:ib * 64 + 64],
                        lhsT=exT[:, jc * 128:(jc + 1) * 128],
                        rhs=vbf[:, jc * 256 + 64 * h:jc * 256 + 64 * h + 64],
                        start=(jc == 0), stop=(jc == 7))
                ctx_sb = sb_small.tile([128, 64], F32, tag="ctx")
                nc.scalar.activation(ctx_sb[:], ctxps[:, ib * 64:ib * 64 + 64],
                                     AF.Copy, scale=rs[:])
                nc.scalar.dma_start(
                    ctx_o[h, ib * 128:(ib + 1) * 128, :], ctx_sb[:])

    nc.compile()
    return nc


# --------------------------------------------------------------------------
# host wrapper
# --------------------------------------------------------------------------

def _numpy_reference(hidden_states, attention_mask, relative_pos,
                     rel_embeddings, Wq, bq, Wk, bk, Wv, bv):
    """Fallback exact path (only used if attention_mask isn't all ones)."""
    hs = hidden_states.astype(np.float64)
    q = (hs @ Wq.T.astype(np.float64) + bq).reshape(B, S, H, D).transpose(0, 2, 1, 3)
    k = (hs @ Wk.T.astype(np.float64) + bk).reshape(B, S, H, D).transpose(0, 2, 1, 3)
    v = (hs @ Wv.T.astype(np.float64) + bv).reshape(B, S, H, D).transpose(0, 2, 1, 3)
    scores = np.einsum("bhid,bhjd->bhij", q, k) * SCALE
    pos_k = (rel_embeddings @ Wk.T + bk).reshape(2 * K, H, D).transpose(1, 0, 2)
    pos_q = (rel_embeddings @ Wq.T + bq).reshape(2 * K, H, D).transpose(1, 0, 2)
    rp = relative_pos.astype(np.int64)
    rows = np.arange(S)[:, None]
    c2p = np.einsum("bhid,hkd->bhik", q, pos_k) * SCALE
    c2p_g = c2p[:, :, rows, np.clip(rp + K, 0, 2 * K - 1)]
    p2c = np.einsum("bhjd,hkd->bhjk", k, pos_q) * SCALE
    p2c_g = p2c[:, :, rows, np.clip(-rp + K, 0, 2 * K - 1)].transpose(0, 1, 3, 2)
    logits = scores + c2p_g + p2c_g
    logits = logits - logits.max(axis=-1, keepdims=True)
    mask = attention_mask.astype(bool)
    neg = np.finfo(np.float32).min
    x = np.where(mask, logits, neg)
    x = x - x.max(axis=-1, keepdims=True)
    e = np.exp(x)
    probs = e / e.sum(axis=-1, keepdims=True)
    probs = np.where(mask, probs, 0.0)
    ctx = np.einsum("bhij,bhjd->bihd", probs, v).reshape(B, S, HID)
    return (ctx.astype(np.float32), probs.astype(np.float32),
            logits.astype(np.float32))


def kernel(hidden_states, attention_mask, relative_pos, rel_embeddings,
           Wq, bq, Wk, bk, Wv, bv):
    global _COMPILED
    hidden_states = np.asarray(hidden_states, dtype=np.float32)
    attention_mask = np.asarray(attention_mask)
    relative_pos = np.asarray(relative_pos)
    rel_embeddings = np.asarray(rel_embeddings, dtype=np.float32)
    Wq, bq = np.asarray(Wq, np.float32), np.asarray(bq, np.float32)
    Wk, bk = np.asarray(Wk, np.float32), np.asarray(bk, np.float32)
    Wv, bv = np.asarray(Wv, np.float32), np.asarray(bv, np.float32)

    if not np.all(attention_mask == 1):
        return _numpy_reference(hidden_states, attention_mask, relative_pos,
                                rel_embeddings, Wq, bq, Wk, bk, Wv, bv)

    fd = _delta_table(np.asarray(relative_pos, dtype=np.int64))
    t = np.arange(EL - 1)
    mc = np.zeros(EL, dtype=np.int64)
    mp = np.zeros(EL, dtype=np.int64)
    mc[:EL - 1] = np.clip(fd[2046 - t] + K, 0, 2 * K - 1)
    mp[:EL - 1] = np.clip(-fd[2046 - t] + K, 0, 2 * K - 1)
    idxc = _wrap_idx(mc)
    idxp = _wrap_idx(mp)
    ident = np.eye(128, dtype=np.float32).astype(ml_dtypes.bfloat16)

    if _COMPILED is None:
        _COMPILED = _build_kernel()
    nc = _COMPILED

    in_maps = []
    for c in range(NCORES):
        b, hg = c // 4, c % 4
        r0 = 256 * hg
        in_maps.append({
            "hsT": np.ascontiguousarray(hidden_states[b].T).astype(ml_dtypes.bfloat16),
            "wqT": np.ascontiguousarray(Wq[r0:r0 + 256, :].T).astype(ml_dtypes.bfloat16),
            "wkT": np.ascontiguousarray(Wk[r0:r0 + 256, :].T).astype(ml_dtypes.bfloat16),
            "wvT": np.ascontiguousarray(Wv[r0:r0 + 256, :].T).astype(ml_dtypes.bfloat16),
            "bqs": np.ascontiguousarray(
                (bq[r0:r0 + 256] * SCALE).reshape(2, 128).T),
            "bk2": np.ascontiguousarray(bk[r0:r0 + 256].reshape(2, 128).T),
            "bv1": np.ascontiguousarray(np.broadcast_to(bv[r0:r0 + 256], (128, 256))),
            "rembT": np.ascontiguousarray(rel_embeddings.T).astype(ml_dtypes.bfloat16),
            "idxc": idxc,
            "idxp": idxp,
            "ident": ident,
        })

    global LAST_RESULT
    LAST_RESULT = run_bass_kernel_spmd(nc, in_maps, list(range(NCORES)))
    res = LAST_RESULT.results

    logits = np.empty((B, H, S, S), dtype=np.float32)
    probs = np.empty((B, H, S, S), dtype=np.float32)
    ctx = np.empty((B, S, HID), dtype=np.float32)
    for c in range(NCORES):
        b, hg = c // 4, c % 4
        for hh in range(HPC):
            g = 4 * hg + hh
            logits[b, g] = res[c]["logits_o"][hh]
            probs[b, g] = res[c]["probs_o"][hh]
            ctx[b, :, g * 64:(g + 1) * 64] = res[c]["ctx_o"][hh]
    return ctx, probs, logits
